# revision 1
# baseline (speedup 1.0000x reference)
"""Trainium2 Bass kernel for CausalRecurrentAttention (B=2,T=2048,C=1024,H=16,S=16).

Sharding: tensor-parallel over channels/heads. Each of the 8 cores owns 128
channels (= 2 attention heads). The recurrent scan runs per-channel via the
DVE tensor_tensor_scan instruction; LayerNorm stats use an AllReduce and the
normalized hybrid is AllGathered so every core can run its heads' attention.
Final Wo projection is row-sharded; partial outputs are summed on the host.
"""
import sys, os, math

for _p in ("/opt/trn_rl_repo", os.path.expanduser("~/.axon_site/_ro/trn_rl_repo")):
    if os.path.isdir(_p):
        if _p not in sys.path:
            sys.path.insert(0, _p)
        break

import numpy as np
import concourse.bass as bass
import concourse.bacc as bacc
import concourse.mybir as mybir
from concourse import tile
from concourse.bass_utils import run_bass_kernel_spmd

FP = mybir.dt.float32
FPR = mybir.dt.float32r
AX = mybir.AluOpType
AF = mybir.ActivationFunctionType

B, T, C, H, S = 2, 2048, 1024, 16, 16
HD = C // H          # 64
EPS = 1e-5
NCORES = 8
CS = C // NCORES     # 128 channels per core
BT = B * T           # 4096
TCH = 512            # t-chunk width
NJ = BT // TCH       # 8
NCH = C // 128       # 8 contraction chunks
NTB = T // TCH       # 4 chunks per batch element

_CACHE = {}


def _build(collectives=True):
    nc = bacc.Bacc("TRN2", target_bir_lowering=False, debug=False, num_devices=NCORES)

    dt_in = {}
    def din(name, shape, dt):
        dt_in[name] = nc.dram_tensor(name, list(shape), dt, kind="ExternalInput")
        return dt_in[name]

    xT = din("xT", (C, BT), FPR)
    wd = din("wd", (C, CS), FPR)
    wx = din("wx", (C, CS), FPR)
    wbc = din("wbc", (C, 2 * S), FPR)
    wq = din("wq", (C, CS), FPR)
    wk = din("wk", (C, CS), FPR)
    wv = din("wv", (C, CS), FPR)
    wo = din("wo", (CS, C), FPR)
    acol = din("acol", (CS, S), FP)
    bd = din("bd", (CS, 1), FP)
    bx = din("bx", (CS, 1), FP)
    bq = din("bq", (CS, 1), FP)
    kscale = din("kscale", (CS, 1), FP)
    kbias = din("kbias", (CS, 1), FP)
    bv = din("bv", (CS, 1), FP)
    gb2 = din("gb2", (2, CS), FPR)
    onesq = din("onesq", (128, 128), FPR)
    sel = din("sel", (2 * S, 2 * S * 128), FPR)
    ident2 = din("ident2", (128, 64), FPR)
    onesc = din("onesc", (128, 1), FPR)
    ident = din("ident", (128, 128), FPR)
    cmask = din("cmask", (128, 2048), FP)
    ones_bt = din("ones_bt", (1, BT), FPR)

    outp = nc.dram_tensor("outp", [C, BT], FP, kind="ExternalOutput")

    with nc.allow_low_precision(reason="fp32r dtype tags"), tile.TileContext(nc) as tc, \
            tc.tile_pool(name="lvla", bufs=1) as lvla:
        # ---------- level-A persistent tiles (small constants + hybrid) ----------
        id_sb = lvla.tile([128, 128], FPR, name="id_sb")
        oq_sb = lvla.tile([128, 128], FPR, name="oq_sb")
        id2_sb = lvla.tile([128, 64], FPR, name="id2_sb")
        oc_sb = lvla.tile([128, 1], FPR, name="oc_sb")
        gb_sb = lvla.tile([2, 128], FPR, name="gb_sb")
        ac_sb = lvla.tile([128, S], FP, name="ac_sb")
        bcol_sb = lvla.tile([128, 6], FP, name="bcol_sb")  # bd,bx,bq,kscale,kbias,bv
        hyb_sb = lvla.tile([128, BT], FPR, name="hyb_sb")

        nc.sync.dma_start(id_sb[:], ident[:])
        nc.sync.dma_start(oq_sb[:], onesq[:])
        nc.sync.dma_start(id2_sb[:], ident2[:])
        nc.sync.dma_start(oc_sb[:], onesc[:])
        nc.sync.dma_start(gb_sb[:], gb2[:])
        nc.sync.dma_start(ac_sb[:], acol[:])
        for i, t_ in enumerate((bd, bx, bq, kscale, kbias, bv)):
            nc.sync.dma_start(bcol_sb[:, i:i + 1], t_[:])
        BD, BX, BQ, KSC, KBI, BV = (bcol_sb[:, i:i + 1] for i in range(6))

        # DRAM bounce buffers for collectives
        with tc.tile_pool(name="dramp", bufs=1, space="DRAM") as dramp:
            st_loc = dramp.tile([1, 2 * BT], FP, name="st_loc")
            st_sum = dramp.tile([1, 2 * BT], FP, name="st_sum")
            hyn_loc = dramp.tile([128, BT], FPR, name="hyn_loc")
            hyn_all = dramp.tile([C, BT], FPR, name="hyn_all")

            # ================= stage 1: delta / x_base / B / C =================
            with tc.tile_pool(name="s1sb", bufs=1) as s1sb:
                dl_sb = s1sb.tile([128, BT], FP, name="dl_sb")   # delta^T
                xb_sb = s1sb.tile([128, BT], FP, name="xb_sb")   # x_base^T
                du_sb = s1sb.tile([128, BT], FP, name="du_sb")   # delta*x_base
                bc_sb = s1sb.tile([2 * S, BT], FPR, name="bc_sb")  # [B_mat; C_mat]^T
                hl_sb = s1sb.tile([128, S], FP, name="hl_sb")    # scan carry
                sel_sb = s1sb.tile([2 * S, 2 * S * 128], FPR, name="sel_sb")
                nc.sync.dma_start(sel_sb[:], sel[:])

                with (
                    tc.tile_pool(name="s1w", bufs=1) as s1w,
                    tc.tile_pool(name="s1x", bufs=9) as s1x,
                    tc.tile_pool(name="s1ps", bufs=2, space="PSUM") as s1ps,
                ):
                    wd_sb = s1w.tile([128, C], FPR, name="wd_sb")
                    wx_sb = s1w.tile([128, C], FPR, name="wx_sb")
                    wbc_sb = s1w.tile([128, NCH * 2 * S], FPR, name="wbc_sb")
                    for k in range(NCH):
                        sl = slice(k * 128, (k + 1) * 128)
                        nc.sync.dma_start(wd_sb[:, sl], wd[sl, :])
                        nc.sync.dma_start(wx_sb[:, sl], wx[sl, :])
                        nc.sync.dma_start(wbc_sb[:, k * 2 * S:(k + 1) * 2 * S], wbc[sl, :])

                    WLD = 2048
                    for half in range(BT // WLD):
                        xt = [s1x.tile([128, WLD], FPR, name=f"xt{k}", tag="xt") for k in range(NCH)]
                        for k in range(NCH):
                            nc.gpsimd.dma_start(xt[k][:], xT[k * 128:(k + 1) * 128,
                                                             half * WLD:(half + 1) * WLD])
                        for j2 in range(WLD // TCH):
                            j0 = half * WLD + j2 * TCH
                            cj = slice(j0, j0 + TCH)
                            xsl = slice(j2 * TCH, (j2 + 1) * TCH)
                            pd = s1ps.tile([128, TCH], FP, name="pd", tag="pd")
                            px = s1ps.tile([128, TCH], FP, name="px", tag="px")
                            pb = s1ps.tile([2 * S, TCH], FP, name="pb", tag="pb")
                            for k in range(NCH):
                                st, sp = (k == 0), (k == NCH - 1)
                                nc.tensor.matmul(pd[:], wd_sb[:, k * 128:(k + 1) * 128], xt[k][:, xsl], start=st, stop=sp)
                                nc.tensor.matmul(px[:], wx_sb[:, k * 128:(k + 1) * 128], xt[k][:, xsl], start=st, stop=sp)
                                nc.tensor.matmul(pb[:], wbc_sb[:, k * 2 * S:(k + 1) * 2 * S], xt[k][:, xsl], start=st, stop=sp)
                            et = s1x.tile([128, TCH], FP, name="et", tag="et")
                            nc.scalar.activation(et[:], pd[:], AF.Exp, bias=BD)
                            nc.vector.tensor_scalar_add(out=et[:], in0=et[:], scalar1=1.0)
                            nc.scalar.activation(dl_sb[:, cj], et[:], AF.Ln)
                            nc.scalar.activation(xb_sb[:, cj], px[:], AF.Identity, bias=BX)
                            nc.scalar.copy(bc_sb[:, cj], pb[:])
                            nc.vector.tensor_tensor(out=du_sb[:, cj], in0=dl_sb[:, cj], in1=xb_sb[:, cj], op=AX.mult)

                # ================= stage 2: recurrent scan =================
                with (
                    tc.tile_pool(name="s2ps", bufs=3, space="PSUM") as s2ps,
                    tc.tile_pool(name="s2py", bufs=2, space="PSUM") as s2py,
                    tc.tile_pool(name="s2pa", bufs=3) as s2pa,
                    tc.tile_pool(name="s2sb", bufs=4) as s2sb,
                    tc.tile_pool(name="s2h", bufs=4) as s2h,
                ):
                    for b in range(B):
                        for jt in range(NTB):
                            c0 = b * T + jt * TCH
                            cj = slice(c0, c0 + TCH)
                            py = s2py.tile([128, TCH], FP, name="py", tag="py")
                            for s in range(S):
                                pB = s2ps.tile([128, TCH], FP, name="pB", tag="pB")
                                pC = s2ps.tile([128, TCH], FP, name="pC", tag="pC")
                                pa = s2pa.tile([128, TCH], FP, name="pa", tag="pa")
                                nc.tensor.matmul(pB[:], sel_sb[:, s * 128:(s + 1) * 128], bc_sb[:, cj], start=True, stop=True)
                                nc.tensor.matmul(pC[:], sel_sb[:, (S + s) * 128:(S + s + 1) * 128], bc_sb[:, cj], start=True, stop=True)
                                nc.scalar.activation(pa[:], dl_sb[:, cj], AF.Exp, scale=ac_sb[:, s:s + 1])
                                inc = s2sb.tile([128, TCH], FP, name="inc", tag="inc")
                                nc.vector.tensor_tensor(out=inc[:], in0=du_sb[:, cj], in1=pB[:], op=AX.mult)
                                h = s2h.tile([128, TCH], FP, name="h", tag="h")
                                init = 0.0 if jt == 0 else hl_sb[:, s:s + 1]
                                nc.vector.tensor_tensor_scan(h[:], pa[:], inc[:], init, op0=AX.mult, op1=AX.add)
                                nc.gpsimd.tensor_copy(hl_sb[:, s:s + 1], h[:, TCH - 1:TCH])
                                hC = s2sb.tile([128, TCH], FPR, name="hC", tag="hC")
                                nc.vector.tensor_tensor(out=hC[:], in0=h[:], in1=pC[:], op=AX.mult)
                                nc.tensor.matmul(py[:], id_sb[:], hC[:], start=(s == 0), stop=(s == S - 1))
                            nc.vector.tensor_tensor(out=hyb_sb[:, cj], in0=xb_sb[:, cj], in1=py[:], op=AX.add)

                # ---- LayerNorm stats (partial over this core's 128 channels) ----
                with (
                    tc.tile_pool(name="s3ps", bufs=2, space="PSUM") as s3ps,
                    tc.tile_pool(name="s3sb", bufs=2) as s3sb,
                ):
                    st_sb = s3sb.tile([1, 2 * BT], FP, name="st_sb")
                    for j in range(NJ):
                        cj = slice(j * TCH, (j + 1) * TCH)
                        hsq = s3sb.tile([128, TCH], FPR, name="hsq", tag="hsq")
                        nc.vector.tensor_tensor(out=hsq[:], in0=hyb_sb[:, cj].bitcast(FP),
                                                in1=hyb_sb[:, cj].bitcast(FP), op=AX.mult)
                        p1 = s3ps.tile([1, TCH], FP, name="p1", tag="p1")
                        p2 = s3ps.tile([1, TCH], FP, name="p2", tag="p2")
                        nc.tensor.matmul(p1[:], oc_sb[:], hyb_sb[:, cj], start=True, stop=True)
                        nc.tensor.matmul(p2[:], oc_sb[:], hsq[:], start=True, stop=True)
                        nc.scalar.copy(st_sb[0:1, cj], p1[:])
                        nc.scalar.copy(st_sb[0:1, BT + j * TCH:BT + (j + 1) * TCH], p2[:])
                    nc.sync.dma_start(st_loc[:], st_sb[:])

            # stage-1/2 SBUF pools closed here (frees delta/xbase/du/h space)
            if collectives:
                nc.gpsimd.collective_compute(
                    "AllReduce", AX.add, replica_groups=[list(range(NCORES))],
                    ins=[st_loc.opt()], outs=[st_sum.opt()])
            else:
                nc.sync.dma_start(st_sum[:], st_loc[:])

            # ================= stage 3: normalize own shard, AllGather =========
            with (
                tc.tile_pool(name="n_sb", bufs=1) as n_sb,
                tc.tile_pool(name="n_tmp", bufs=3) as n_tmp,
                tc.tile_pool(name="n_ps", bufs=2, space="PSUM") as n_ps,
            ):
                st2 = n_sb.tile([1, 2 * BT], FP, name="st2")
                nc.sync.dma_start(st2[:], st_sum[:])
                sq = n_sb.tile([1, BT], FP, name="sq")
                s2c = n_sb.tile([1, BT], FP, name="s2c")
                varn = n_sb.tile([1, BT], FP, name="varn")
                lvar = n_sb.tile([1, BT], FP, name="lvar")
                rstd = n_sb.tile([1, BT], FPR, name="rstd")
                nmr2 = n_sb.tile([2, BT], FPR, name="nmr2")
                nc.vector.tensor_tensor(out=sq[:], in0=st2[0:1, 0:BT], in1=st2[0:1, 0:BT], op=AX.mult)
                nc.scalar.mul(s2c[:], st2[0:1, BT:2 * BT], 1.0 / C)
                nc.vector.scalar_tensor_tensor(out=varn[:], in0=sq[:], scalar=-1.0 / (C * C),
                                               in1=s2c[:], op0=AX.mult, op1=AX.add)
                nc.vector.tensor_scalar_add(out=varn[:], in0=varn[:], scalar1=float(EPS))
                nc.scalar.activation(lvar[:], varn[:], AF.Ln)
                nc.scalar.activation(rstd[:], lvar[:], AF.Exp, scale=-0.5)
                nc.sync.dma_start(nmr2[1:2, :], ones_bt[:])
                nc.vector.scalar_tensor_tensor(out=nmr2[0:1, :], in0=st2[0:1, 0:BT], scalar=-1.0 / C,
                                               in1=rstd[:].bitcast(FP), op0=AX.mult, op1=AX.mult)
                for j in range(NJ):
                    cj = slice(j * TCH, (j + 1) * TCH)
                    pr = n_ps.tile([128, TCH], FP, name="pr", tag="pr")
                    pn = n_ps.tile([128, TCH], FP, name="pn", tag="pn")
                    nc.tensor.matmul(pr[:], oq_sb[0:1, :], rstd[:, cj], start=True, stop=True)
                    nc.tensor.matmul(pn[:], gb_sb[:], nmr2[:, cj], start=True, stop=True)
                    f1 = n_tmp.tile([128, TCH], FP, name="f1", tag="f1")
                    nc.vector.tensor_tensor(out=f1[:], in0=hyb_sb[:, cj].bitcast(FP), in1=pr[:], op=AX.mult)
                    hn = n_tmp.tile([128, TCH], FPR, name="hn", tag="hn")
                    nc.vector.tensor_tensor(out=hn[:], in0=f1[:], in1=pn[:], op=AX.add)
                    nc.sync.dma_start(hyn_loc[:, cj], hn[:])

            if collectives:
                nc.gpsimd.collective_compute(
                    "AllGather", AX.bypass, replica_groups=[list(range(NCORES))],
                    ins=[hyn_loc.opt()], outs=[hyn_all.opt()])
            else:
                for _c in range(NCORES):
                    nc.sync.dma_start(hyn_all[_c * 128:(_c + 1) * 128, :], hyn_loc[:])

            # ================= stage 4: Q/K/V projections ======================
            with tc.tile_pool(name="lvlb", bufs=1) as lvlb:
                with (
                    tc.tile_pool(name="s4w", bufs=1) as s4w,
                    tc.tile_pool(name="s4vt", bufs=1) as s4vt,
                    tc.tile_pool(name="s4x", bufs=10) as s4x,
                    tc.tile_pool(name="s4ps", bufs=2, space="PSUM") as s4ps,
                    tc.tile_pool(name="s4tp", bufs=2, space="PSUM") as s4tp,
                ):
                    wo_sb = lvlb.tile([128, C], FPR, name="wo_sb")
                    cm_sb = lvlb.tile([128, 2048], FP, name="cm_sb")
                    qt_sb = lvlb.tile([128, BT], FPR, name="qt_sb")
                    kt_sb = lvlb.tile([128, BT], FPR, name="kt_sb")
                    v_sb = lvlb.tile([128, B * 2 * (T // 128) * 65], FPR, name="v_sb")
                    at_sb = lvlb.tile([128, BT], FPR, name="at_sb")
                    nc.sync.dma_start(wo_sb[:], wo[:])
                    nc.sync.dma_start(cm_sb[:], cmask[:])
                    nc.gpsimd.memset(v_sb[:].bitcast(FP), 1.0)
                    wq_sb = s4w.tile([128, C], FPR, name="wq_sb")
                    wk_sb = s4w.tile([128, C], FPR, name="wk_sb")
                    wv_sb = s4w.tile([128, C], FPR, name="wv_sb")
                    for k in range(NCH):
                        sl = slice(k * 128, (k + 1) * 128)
                        nc.sync.dma_start(wq_sb[:, sl], wq[sl, :])
                        nc.sync.dma_start(wk_sb[:, sl], wk[sl, :])
                        nc.sync.dma_start(wv_sb[:, sl], wv[sl, :])
                    vt_sb = s4vt.tile([128, BT], FPR, name="vt_sb")
                    WH = 1024
                    for half in range(BT // WH):
                        hx = [s4x.tile([128, WH], FPR, name=f"hx{k}", tag="hx") for k in range(NCH)]
                        for k in range(NCH):
                            nc.sync.dma_start(hx[k][:], hyn_all[k * 128:(k + 1) * 128,
                                                                half * WH:(half + 1) * WH])
                        for j2 in range(WH // TCH):
                            j0 = half * WH + j2 * TCH
                            cj = slice(j0, j0 + TCH)
                            xsl = slice(j2 * TCH, (j2 + 1) * TCH)
                            pq = s4ps.tile([128, TCH], FP, name="pq", tag="pq")
                            pk = s4ps.tile([128, TCH], FP, name="pk", tag="pk")
                            pv = s4ps.tile([128, TCH], FP, name="pv", tag="pv")
                            for k in range(NCH):
                                st, sp = (k == 0), (k == NCH - 1)
                                nc.tensor.matmul(pq[:], wq_sb[:, k * 128:(k + 1) * 128], hx[k][:, xsl], start=st, stop=sp)
                                nc.tensor.matmul(pk[:], wk_sb[:, k * 128:(k + 1) * 128], hx[k][:, xsl], start=st, stop=sp)
                                nc.tensor.matmul(pv[:], wv_sb[:, k * 128:(k + 1) * 128], hx[k][:, xsl], start=st, stop=sp)
                            nc.scalar.activation(qt_sb[:, cj], pq[:], AF.Identity, bias=BQ)
                            nc.scalar.activation(kt_sb[:, cj], pk[:], AF.Identity, scale=KSC, bias=KBI)
                            nc.scalar.activation(vt_sb[:, cj], pv[:], AF.Identity, bias=BV)
                    # transpose V^T -> V blocks [128t, 64d] (+ones col at 64)
                    for b in range(B):
                        for h in range(2):
                            for kt in range(T // 128):
                                blk = ((b * 2 + h) * (T // 128) + kt) * 65
                                tp = s4tp.tile([128, 64], FPR, name="tp", tag="tp")
                                nc.tensor.transpose(
                                    tp[:], vt_sb[64 * h:64 * h + 64, b * T + kt * 128: b * T + (kt + 1) * 128],
                                    id2_sb[64 * h:64 * h + 64, :])
                                nc.scalar.copy(v_sb[:, blk:blk + 64], tp[:])

                # ================= stage 5: attention ==============================
                with (
                    tc.tile_pool(name="s5p", bufs=6) as s5p,
                    tc.tile_pool(name="s5o", bufs=2) as s5o,
                    tc.tile_pool(name="s5ps", bufs=4, space="PSUM") as s5ps,
                    tc.tile_pool(name="s5po", bufs=2, space="PSUM") as s5po,
                    tc.tile_pool(name="s5pr", bufs=1, space="PSUM") as s5pr,
                ):
                    for b in range(B):
                        for h in range(2):
                            hsl = slice(64 * h, 64 * h + 64)
                            for qc in range(T // TCH):
                                q0 = b * T + qc * TCH
                                po = s5po.tile([65, TCH], FP, name="po", tag="po")
                                nkb = (qc + 1) * (TCH // 128)
                                for kb in range(nkb):
                                    ps = s5ps.tile([128, TCH], FP, name="ps", tag="ps")
                                    nc.tensor.matmul(
                                        ps[:], kt_sb[hsl, b * T + kb * 128: b * T + (kb + 1) * 128],
                                        qt_sb[hsl, q0:q0 + TCH], start=True, stop=True)
                                    pt = s5p.tile([128, TCH], FPR, name="pt", tag="pt")
                                    nc.scalar.activation(pt[:], ps[:], AF.Exp)
                                    d = kb - qc * (TCH // 128)
                                    if d >= 0:
                                        # quarters left of the diagonal sub-block are fully
                                        # masked; the diagonal one needs the staircase mask
                                        if d > 0:
                                            nc.gpsimd.memset(pt[:, 0:d * 128].bitcast(FP), 0.0)
                                        nc.vector.tensor_tensor(
                                            out=pt[:, d * 128:(d + 1) * 128],
                                            in0=pt[:, d * 128:(d + 1) * 128].bitcast(FP),
                                            in1=cm_sb[:, 0:128], op=AX.mult)
                                    blk = ((b * 2 + h) * (T // 128) + kb) * 65
                                    nc.tensor.matmul(po[:], v_sb[:, blk:blk + 65], pt[:],
                                                     start=(kb == 0), stop=(kb == nkb - 1))
                                rt = s5o.tile([65, TCH], FPR, name="rt", tag="rt")
                                nc.vector.reciprocal(rt[64:65, :], po[64:65, :])
                                pr = s5pr.tile([64, TCH], FP, name="prr", tag="prr")
                                nc.tensor.matmul(pr[:], oq_sb[64:65, 0:64], rt[64:65, :], start=True, stop=True)
                                ot = s5o.tile([64, TCH], FP, name="ot", tag="ot")
                                nc.scalar.copy(ot[:], po[0:64, :])
                                nc.vector.tensor_tensor(out=at_sb[hsl, q0:q0 + TCH], in0=ot[:],
                                                        in1=pr[:], op=AX.mult)

                # ================= stage 6: Wo partial =============================
                with (
                    tc.tile_pool(name="s6o", bufs=2) as s6o,
                    tc.tile_pool(name="s6ps", bufs=4, space="PSUM") as s6ps,
                ):
                    for oc in range(NCH):
                        ob = s6o.tile([128, BT], FP, name="ob", tag="ob")
                        for j in range(NJ):
                            cj = slice(j * TCH, (j + 1) * TCH)
                            pso = s6ps.tile([128, TCH], FP, name="pso", tag="pso")
                            nc.tensor.matmul(pso[:], wo_sb[:, oc * 128:(oc + 1) * 128],
                                             at_sb[:, cj], start=True, stop=True)
                            if j % 2 == 0:
                                nc.scalar.copy(ob[:, cj], pso[:])
                            else:
                                nc.vector.tensor_copy(ob[:, cj], pso[:])
                        nc.gpsimd.dma_start(outp[oc * 128:(oc + 1) * 128, :], ob[:])

    nc.compile()
    return nc


def _softplus(v):
    return np.log1p(np.exp(-np.abs(v))) + np.maximum(v, 0.0)


_SEL = np.zeros((2 * S, 2 * S * 128), np.float32)
for _i in range(2 * S):
    _SEL[_i, _i * 128:(_i + 1) * 128] = 1.0


def _prep_inputs(x, A_log, Wd, bd, WB, WC, Wq, bq, Wk, bk, Wv, bv, Wx, bx,
                 Wo, bo, ln_g, ln_b, temp):
    f32 = np.float32
    xT = np.ascontiguousarray(np.asarray(x, f32).reshape(BT, C).T)
    A = -np.exp(np.asarray(A_log, f32))
    wbc = np.concatenate([np.asarray(WB, f32), np.asarray(WC, f32)], axis=1)
    cmask = np.zeros((128, 4 * TCH), f32)
    for d in range(4):
        p = np.arange(128)[:, None] + 128 * d
        f = np.arange(TCH)[None, :]
        cmask[:, d * TCH:(d + 1) * TCH] = (f >= p).astype(f32)
    sc = np.asarray(temp, f32).reshape(H)  # per-head temp
    sc = _softplus(sc) / math.sqrt(HD)

    in_maps = []
    for cid in range(NCORES):
        sl = slice(cid * CS, (cid + 1) * CS)
        heads = [2 * cid, 2 * cid + 1]
        kcol = np.repeat(sc[heads], HD).astype(f32)[:, None]          # (128,1)
        im = {
            "xT": xT,
            "wd": np.ascontiguousarray(np.asarray(Wd, f32)[:, sl]),
            "wx": np.ascontiguousarray(np.asarray(Wx, f32)[:, sl]),
            "wbc": wbc,
            "wq": np.ascontiguousarray(np.asarray(Wq, f32)[:, sl]),
            "wk": np.ascontiguousarray(np.asarray(Wk, f32)[:, sl]),
            "wv": np.ascontiguousarray(np.asarray(Wv, f32)[:, sl]),
            "wo": np.ascontiguousarray(np.asarray(Wo, f32)[sl, :]),
            "acol": np.ascontiguousarray(A[sl]),
            "bd": np.asarray(bd, f32)[sl][:, None],
            "bx": np.asarray(bx, f32)[sl][:, None],
            "bq": np.asarray(bq, f32)[sl][:, None],
            "kscale": kcol,
            "kbias": (np.asarray(bk, f32)[sl][:, None] * kcol).astype(f32),
            "bv": np.asarray(bv, f32)[sl][:, None],
            "gb2": np.stack([np.asarray(ln_g, f32)[sl], np.asarray(ln_b, f32)[sl]]),
            "onesq": np.ones((128, 128), f32),
            "onesc": np.ones((128, 1), f32),
            "ident": np.eye(128, dtype=f32),
            "ident2": np.vstack([np.eye(64, dtype=f32)] * 2),
            "sel": _SEL,
            "cmask": cmask,
            "ones_bt": np.ones((1, BT), f32),
        }
        im = {k: np.ascontiguousarray(v, dtype=f32) for k, v in im.items()}
        in_maps.append(im)
    return in_maps


def kernel(**inputs):
    if "nc" not in _CACHE:
        _CACHE["nc"] = _build()
    nc = _CACHE["nc"]
    in_maps = _prep_inputs(**inputs)
    res = run_bass_kernel_spmd(nc, in_maps, core_ids=list(range(NCORES)))
    total = np.zeros((C, BT), np.float64)
    for r in res.results:
        total += r["outp"]
    out = total.T.reshape(B, T, C) + np.asarray(inputs["bo"], np.float64)[None, None, :]
    return out.astype(np.float32)

